# revision 6
# baseline (speedup 1.0000x reference)
"""CenterCTCLoss Trainium2 kernel.

Strategy (data-parallel over batch, 8 cores, 512 rows each):
  The reference computation collapses to three segment statistics per core:
    counts[c] = sum of mask where labels==c
    S[c,d]    = sum of mask*features where labels==c
    SQ[c,d]   = sum of mask*features^2 where labels==c   (q = SQ.sum())
  where mask is the CTC char mask derived from argmax(preds).  Everything
  else (loss, centers update) is O(n_class*feat) math done on host from the
  8 per-core partial sums:
    upd[c]      = ALPHA*(counts[c]*centers[c] - S[c]) / (1+counts[c])
    new_centers = centers - upd
    loss        = 0.5*(sum_c counts[c]*|centers[c]|^2 - 2*<centers,S> + q)

  On device per chunk of 128 batch rows (batch on partitions):
    m[b,t]   = reduce_max over classes            (DVE)
    eq       = (preds == m)  exact one-hot        (DVE)
    sh       = eq[t]*eq[t+1] shifted product      (GPSIMD)
    rep[b,t] = sum_c sh                           (DVE)  == 1 iff argmax repeats
    mask     = (1-eq[...,BLANK])*(1-rep)          (tiny)
    moh      = one-hot(labels + 85*(1-mask)) bf16 (DVE)  masked one-hot
    psum[85,129] += moh_t.T @ [x_t | x_t^2 | 1]   (PE, bf16 ops, fp32 accum)
"""

import sys

sys.path.insert(0, "/opt/trn_rl_repo")

from contextlib import ExitStack

import numpy as np

import concourse.bacc as bacc
import concourse.bass as bass
import concourse.mybir as mybir
import concourse.tile as tile
from concourse.bass_utils import run_bass_kernel_spmd

N_CLASS = 85
BLANK = 84
ALPHA = 0.05
B, T, D = 4096, 96, 64
NCORES = 8
BS = B // NCORES  # 512 batch rows per core
TH = T // 2  # half-chunk along t for DMA tiles

f32 = mybir.dt.float32
i32 = mybir.dt.int32
i16 = mybir.dt.int16
bf16 = mybir.dt.bfloat16

FDP = T * N_CLASS  # 8160 preds elems per row
NOUT = 2 * D + 1  # 129 output cols: S | SQ | counts


def build_program(bs: int = BS):
    nchunk = bs // 128
    nc = bacc.Bacc(
        "TRN2", target_bir_lowering=False, debug=False, num_devices=NCORES
    )
    preds = nc.dram_tensor("preds", [bs, T, N_CLASS], f32, kind="ExternalInput").ap()
    feats = nc.dram_tensor("features", [bs, T, D], f32, kind="ExternalInput").ap()
    labels = nc.dram_tensor("labels", [bs, T], i32, kind="ExternalInput").ap()
    out = nc.dram_tensor("out", [N_CLASS, NOUT], f32, kind="ExternalOutput").ap()

    with tile.TileContext(nc) as tc, ExitStack() as ctx:
        consts = ctx.enter_context(tc.tile_pool(name="consts", bufs=1))
        ppool = ctx.enter_context(tc.tile_pool(name="ppool", bufs=2))
        xpool = ctx.enter_context(tc.tile_pool(name="xpool", bufs=2))
        work = ctx.enter_context(tc.tile_pool(name="work", bufs=1))
        small = ctx.enter_context(tc.tile_pool(name="small", bufs=2))
        psum_pool = ctx.enter_context(
            tc.tile_pool(name="psum", bufs=1, space="PSUM")
        )

        # iota over classes, repeating per t: iotaC[p, t*85+c] = c
        iotaC = consts.tile([128, FDP], i16)
        nc.gpsimd.iota(
            iotaC[:], pattern=[[0, T], [1, N_CLASS]], base=0, channel_multiplier=0
        )

        psumA = psum_pool.tile([N_CLASS, D], f32)  # S
        psumB = psum_pool.tile([N_CLASS, D + 1], f32)  # SQ | counts

        for c in range(nchunk):
            brange = slice(c * 128, (c + 1) * 128)

            # ---- load + argmax-max per half (pipelined by tile pools) ----
            eq = work.tile([128, FDP], f32, tag="eq")
            eq3 = eq[:].rearrange("p (t c) -> p t c", c=N_CLASS)
            m = small.tile([128, T], f32, tag="m")
            xb = work.tile([128, T, D], bf16, tag="xb")
            xsq = work.tile([128, T, D + 1], bf16, tag="xsq")

            for h in range(2):
                ts_ = slice(h * TH, (h + 1) * TH)
                ph = ppool.tile([128, TH, N_CLASS], f32, tag="ph")
                nc.gpsimd.dma_start(ph[:], preds[brange, ts_, :])
                xh = xpool.tile([128, TH, D], f32, tag="xh")
                nc.gpsimd.dma_start(xh[:], feats[brange, ts_, :])

                nc.vector.tensor_reduce(
                    m[:, ts_], ph[:], axis=mybir.AxisListType.X, op=mybir.AluOpType.max
                )
                mb = m[:, ts_].unsqueeze(2).broadcast_to([128, TH, N_CLASS])
                nc.vector.tensor_tensor(
                    eq3[:, ts_, :], ph[:], mb, op=mybir.AluOpType.is_equal
                )
                # features: bf16 copy and squares (ScalarE)
                nc.scalar.copy(xb[:, ts_, :], xh[:])
                nc.scalar.square(xsq[:, ts_, 0:D], xh[:])

            nc.vector.memset(xsq[:, :, D], 1.0)

            labI = small.tile([128, T], i32, tag="labI")
            nc.gpsimd.dma_start(labI[:], labels[brange, :])

            # ---- repeated-argmax detection ----
            sh = work.tile([128, (T - 1) * N_CLASS], bf16, tag="sh")
            nc.gpsimd.tensor_mul(
                sh[:], eq[:, 0 : (T - 1) * N_CLASS], eq[:, N_CLASS:FDP]
            )
            rep = small.tile([128, T], f32, tag="rep")
            nc.vector.tensor_reduce(
                rep[:, 0 : T - 1],
                sh[:].rearrange("p (t c) -> p t c", c=N_CLASS),
                axis=mybir.AxisListType.X,
                op=mybir.AluOpType.add,
            )
            nc.vector.memset(rep[:, T - 1 : T], 0.0)

            # ---- mask = (1 - eq[..,BLANK]) * (1 - rep) ----
            inv84 = small.tile([128, T], f32, tag="inv84")
            nc.vector.tensor_scalar(
                inv84[:], eq3[:, :, BLANK], -1.0, 1.0,
                op0=mybir.AluOpType.mult, op1=mybir.AluOpType.add,
            )
            invrep = small.tile([128, T], f32, tag="invrep")
            nc.vector.tensor_scalar(
                invrep[:], rep[:], -1.0, 1.0,
                op0=mybir.AluOpType.mult, op1=mybir.AluOpType.add,
            )
            mask = small.tile([128, T], f32, tag="mask")
            nc.vector.tensor_mul(mask[:], invrep[:], inv84[:])

            # ---- masked labels: lab' = labels + 85*(1-mask) (int16) ----
            labf = small.tile([128, T], f32, tag="labf")
            nc.scalar.copy(labf[:], labI[:])
            mterm = small.tile([128, T], f32, tag="mterm")
            nc.vector.tensor_scalar(
                mterm[:], mask[:], -float(N_CLASS), float(N_CLASS),
                op0=mybir.AluOpType.mult, op1=mybir.AluOpType.add,
            )
            labm = small.tile([128, T], i16, tag="labm")
            nc.vector.tensor_add(labm[:], mterm[:], labf[:])

            # ---- masked one-hot (bf16): moh[p,t,c] = (iotaC == lab') ----
            moh = work.tile([128, FDP], bf16, tag="moh")
            moh3 = moh[:].rearrange("p (t c) -> p t c", c=N_CLASS)
            lb = labm[:].unsqueeze(2).broadcast_to([128, T, N_CLASS])
            nc.vector.tensor_tensor(
                moh3[:, :, :],
                iotaC[:].rearrange("p (t c) -> p t c", c=N_CLASS),
                lb,
                op=mybir.AluOpType.is_equal,
            )

            # ---- segment sums via PE ----
            for t in range(T):
                first = c == 0 and t == 0
                last = c == nchunk - 1 and t == T - 1
                nc.tensor.matmul(
                    psumA[:],
                    moh3[:, t, :],
                    xb[:, t, :],
                    start=first,
                    stop=last,
                )
                nc.tensor.matmul(
                    psumB[:],
                    moh3[:, t, :],
                    xsq[:, t, :],
                    start=first,
                    stop=last,
                )

        outT = consts.tile([N_CLASS, NOUT], f32)
        nc.scalar.copy(outT[:, 0:D], psumA[:])
        nc.scalar.copy(outT[:, D : 2 * D + 1], psumB[:])
        nc.gpsimd.dma_start(out[:], outT[:])

    nc.compile()
    return nc


_prog_cache: dict[int, object] = {}


def _get_program(bs: int = BS):
    if bs not in _prog_cache:
        _prog_cache[bs] = build_program(bs)
    return _prog_cache[bs]


def combine_partials(parts: np.ndarray, centers: np.ndarray):
    """parts: [ncores, N_CLASS, NOUT] fp32 device partials -> (loss, new_centers)."""
    tot = parts.astype(np.float64).sum(axis=0)
    S = tot[:, 0:D]
    SQ = tot[:, D : 2 * D]
    counts = tot[:, 2 * D]
    q = SQ.sum()
    c64 = centers.astype(np.float64)
    upd = ALPHA * (counts[:, None] * c64 - S) / (1.0 + counts[:, None])
    new_centers = (c64 - upd).astype(np.float32)
    loss = 0.5 * ((counts * (c64**2).sum(1)).sum() - 2.0 * (c64 * S).sum() + q)
    return np.float32(loss), new_centers


def kernel(preds, features, labels, centers):
    nc = _get_program()
    in_maps = []
    for k in range(NCORES):
        sl = slice(k * BS, (k + 1) * BS)
        in_maps.append(
            {
                "preds": np.ascontiguousarray(preds[sl], dtype=np.float32),
                "features": np.ascontiguousarray(features[sl], dtype=np.float32),
                "labels": np.ascontiguousarray(labels[sl], dtype=np.int32),
            }
        )
    res = run_bass_kernel_spmd(nc, in_maps, core_ids=list(range(NCORES)))
    parts = np.stack([res.results[k]["out"] for k in range(NCORES)])
    return combine_partials(parts, np.asarray(centers, dtype=np.float32))


# revision 9
# speedup vs baseline: 1.1425x; 1.1425x over previous
"""CenterCTCLoss Trainium2 kernel.

Strategy (data-parallel over batch, 8 cores, 512 rows each):
  The reference computation collapses to three segment statistics per core:
    counts[c] = sum of mask where labels==c
    S[c,d]    = sum of mask*features where labels==c
    SQ[c,d]   = sum of mask*features^2 where labels==c   (q = SQ.sum())
  where mask is the CTC char mask derived from argmax(preds).  Everything
  else (loss, centers update) is O(n_class*feat) math done on host from the
  8 per-core partial sums:
    upd[c]      = ALPHA*(counts[c]*centers[c] - S[c]) / (1+counts[c])
    new_centers = centers - upd
    loss        = 0.5*(sum_c counts[c]*|centers[c]|^2 - 2*<centers,S> + q)

  On device per chunk of 128 batch rows (batch on partitions):
    m[b,t]   = reduce_max over classes            (DVE)
    eq       = (preds == m)  exact one-hot        (DVE)
    sh       = eq[t]*eq[t+1] shifted product      (GPSIMD)
    rep[b,t] = sum_c sh                           (DVE)  == 1 iff argmax repeats
    mask     = (1-eq[...,BLANK])*(1-rep)          (tiny)
    moh      = one-hot(labels + 85*(1-mask)) bf16 (DVE)  masked one-hot
    psum[85,129] += moh_t.T @ [x_t | x_t^2 | 1]   (PE, bf16 ops, fp32 accum)
"""

import sys

sys.path.insert(0, "/opt/trn_rl_repo")

from contextlib import ExitStack

import numpy as np

import concourse.bacc as bacc
import concourse.bass as bass
import concourse.mybir as mybir
import concourse.tile as tile
from concourse.bass_utils import run_bass_kernel_spmd

N_CLASS = 85
BLANK = 84
ALPHA = 0.05
B, T, D = 4096, 96, 64
NCORES = 8
BS = B // NCORES  # 512 batch rows per core
TH = T // 2  # half-chunk along t for DMA tiles

f32 = mybir.dt.float32
i32 = mybir.dt.int32
i16 = mybir.dt.int16
bf16 = mybir.dt.bfloat16

FDP = T * N_CLASS  # 8160 preds elems per row
NOUT = 2 * D + 1  # 129 output cols: S | SQ | counts


def build_program(bs: int = BS):
    nchunk = bs // 128
    nc = bacc.Bacc(
        "TRN2", target_bir_lowering=False, debug=False, num_devices=NCORES
    )
    preds = nc.dram_tensor("preds", [bs, T, N_CLASS], f32, kind="ExternalInput").ap()
    feats = nc.dram_tensor("features", [bs, T, D], f32, kind="ExternalInput").ap()
    labels = nc.dram_tensor("labels", [bs, T], i32, kind="ExternalInput").ap()
    out = nc.dram_tensor("out", [N_CLASS, NOUT], f32, kind="ExternalOutput").ap()

    CP = N_CLASS + 1  # 86: class dim padded so bf16 shifted views stay 4B-aligned

    with tile.TileContext(nc) as tc, ExitStack() as ctx:
        consts = ctx.enter_context(tc.tile_pool(name="consts", bufs=1))
        ppool = ctx.enter_context(tc.tile_pool(name="ppool", bufs=2))
        xpool = ctx.enter_context(tc.tile_pool(name="xpool", bufs=2))
        work = ctx.enter_context(tc.tile_pool(name="work", bufs=1))
        mmin = ctx.enter_context(tc.tile_pool(name="mmin", bufs=2))
        small = ctx.enter_context(tc.tile_pool(name="small", bufs=2))
        psum_pool = ctx.enter_context(
            tc.tile_pool(name="psum", bufs=1, space="PSUM")
        )

        # iota over classes, repeating per t: iotaC[p, t*85+c] = c
        iotaC = consts.tile([128, FDP], i16)
        nc.gpsimd.iota(
            iotaC[:], pattern=[[0, T], [1, N_CLASS]], base=0, channel_multiplier=0
        )

        psumA = psum_pool.tile([N_CLASS, D], f32)  # S
        psumB = psum_pool.tile([N_CLASS, D + 1], f32)  # SQ | counts

        for c in range(nchunk):
            brange = slice(c * 128, (c + 1) * 128)

            # ---- load; dd[p,t,c] = preds - max <= 0, exactly 0 at argmax ----
            dd = work.tile([128, T, CP], bf16, tag="dd")
            ddf = dd[:].rearrange("p t c -> p (t c)")
            m = small.tile([128, T], f32, tag="m")
            mn = small.tile([128, T], f32, tag="mn")
            xb = mmin.tile([128, T, D], bf16, tag="xb")
            xsq = mmin.tile([128, T, D + 1], bf16, tag="xsq")

            nc.vector.memset(dd[:, :, N_CLASS], -1e30)
            for h in range(2):
                ts_ = slice(h * TH, (h + 1) * TH)
                ph = ppool.tile([128, TH, N_CLASS], f32, tag="ph")
                nc.gpsimd.dma_start(ph[:], preds[brange, ts_, :])
                xh = xpool.tile([128, TH, D], f32, tag="xh")
                nc.gpsimd.dma_start(xh[:], feats[brange, ts_, :])

                nc.vector.tensor_reduce(
                    m[:, ts_], ph[:], axis=mybir.AxisListType.X, op=mybir.AluOpType.max
                )
                nc.vector.tensor_scalar_mul(mn[:, ts_], m[:, ts_], -1.0)
                mb = mn[:, ts_].unsqueeze(2).broadcast_to([128, TH, N_CLASS])
                nc.gpsimd.tensor_tensor(
                    dd[:, ts_, 0:N_CLASS], ph[:], mb, op=mybir.AluOpType.add
                )
                # features: bf16 copy and squares (ScalarE)
                nc.scalar.copy(xb[:, ts_, :], xh[:])
                nc.scalar.square(xsq[:, ts_, 0:D], xh[:])

            nc.vector.memset(xsq[:, :, D], 1.0)

            labI = small.tile([128, T], i32, tag="labI")
            nc.gpsimd.dma_start(labI[:], labels[brange, :])

            # ---- repeated argmax: max_c(dd_t + dd_{t+1}) == 0 (exact) ----
            dsum = work.tile([128, (T - 1) * CP], bf16, tag="dsum")
            nc.vector.tensor_add(
                dsum[:], ddf[:, 0 : (T - 1) * CP], ddf[:, CP : T * CP]
            )
            zmax = small.tile([128, T], f32, tag="zmax")
            nc.vector.tensor_reduce(
                zmax[:, 0 : T - 1],
                dsum[:].rearrange("p (t c) -> p t c", c=CP),
                axis=mybir.AxisListType.X,
                op=mybir.AluOpType.max,
            )
            nc.vector.memset(zmax[:, T - 1 : T], -1.0)

            # ---- mask = (dd[..,BLANK] != 0) * (zmax != 0) ----
            inv84 = small.tile([128, T], f32, tag="inv84")
            nc.vector.tensor_single_scalar(
                inv84[:], dd[:, :, BLANK], 0.0, op=mybir.AluOpType.not_equal
            )
            invrep = small.tile([128, T], f32, tag="invrep")
            nc.vector.tensor_single_scalar(
                invrep[:], zmax[:], 0.0, op=mybir.AluOpType.not_equal
            )
            mask = small.tile([128, T], f32, tag="mask")
            nc.vector.tensor_mul(mask[:], invrep[:], inv84[:])

            # ---- masked labels: lab' = labels + 85*(1-mask) (int16) ----
            labf = small.tile([128, T], f32, tag="labf")
            nc.scalar.copy(labf[:], labI[:])
            mterm = small.tile([128, T], f32, tag="mterm")
            nc.vector.tensor_scalar(
                mterm[:], mask[:], -float(N_CLASS), float(N_CLASS),
                op0=mybir.AluOpType.mult, op1=mybir.AluOpType.add,
            )
            labm = small.tile([128, T], i16, tag="labm")
            nc.vector.tensor_add(labm[:], mterm[:], labf[:])

            # ---- masked one-hot (bf16): moh[p,t,c] = (iotaC == lab') ----
            moh = work.tile([128, FDP], bf16, tag="moh")
            moh3 = moh[:].rearrange("p (t c) -> p t c", c=N_CLASS)
            lb = labm[:].unsqueeze(2).broadcast_to([128, T, N_CLASS])
            nc.vector.tensor_tensor(
                moh3[:, :, :],
                iotaC[:].rearrange("p (t c) -> p t c", c=N_CLASS),
                lb,
                op=mybir.AluOpType.is_equal,
            )

            # ---- segment sums via PE ----
            for t in range(T):
                first = c == 0 and t == 0
                last = c == nchunk - 1 and t == T - 1
                nc.tensor.matmul(
                    psumA[:],
                    moh3[:, t, :],
                    xb[:, t, :],
                    start=first,
                    stop=last,
                )
                nc.tensor.matmul(
                    psumB[:],
                    moh3[:, t, :],
                    xsq[:, t, :],
                    start=first,
                    stop=last,
                )

        outT = consts.tile([N_CLASS, NOUT], f32)
        nc.scalar.copy(outT[:, 0:D], psumA[:])
        nc.scalar.copy(outT[:, D : 2 * D + 1], psumB[:])
        nc.gpsimd.dma_start(out[:], outT[:])

    nc.compile()
    return nc


_prog_cache: dict[int, object] = {}


def _get_program(bs: int = BS):
    if bs not in _prog_cache:
        _prog_cache[bs] = build_program(bs)
    return _prog_cache[bs]


def combine_partials(parts: np.ndarray, centers: np.ndarray):
    """parts: [ncores, N_CLASS, NOUT] fp32 device partials -> (loss, new_centers)."""
    tot = parts.astype(np.float64).sum(axis=0)
    S = tot[:, 0:D]
    SQ = tot[:, D : 2 * D]
    counts = tot[:, 2 * D]
    q = SQ.sum()
    c64 = centers.astype(np.float64)
    upd = ALPHA * (counts[:, None] * c64 - S) / (1.0 + counts[:, None])
    new_centers = (c64 - upd).astype(np.float32)
    loss = 0.5 * ((counts * (c64**2).sum(1)).sum() - 2.0 * (c64 * S).sum() + q)
    return np.float32(loss), new_centers


def kernel(preds, features, labels, centers):
    nc = _get_program()
    in_maps = []
    for k in range(NCORES):
        sl = slice(k * BS, (k + 1) * BS)
        in_maps.append(
            {
                "preds": np.ascontiguousarray(preds[sl], dtype=np.float32),
                "features": np.ascontiguousarray(features[sl], dtype=np.float32),
                "labels": np.ascontiguousarray(labels[sl], dtype=np.int32),
            }
        )
    res = run_bass_kernel_spmd(nc, in_maps, core_ids=list(range(NCORES)))
    parts = np.stack([res.results[k]["out"] for k in range(NCORES)])
    return combine_partials(parts, np.asarray(centers, dtype=np.float32))


# revision 11
# speedup vs baseline: 1.1492x; 1.0058x over previous
"""CenterCTCLoss Trainium2 kernel.

Strategy (data-parallel over batch, 8 cores, 512 rows each):
  The reference computation collapses to three segment statistics per core:
    counts[c] = sum of mask where labels==c
    S[c,d]    = sum of mask*features where labels==c
    SQ[c,d]   = sum of mask*features^2 where labels==c   (q = SQ.sum())
  where mask is the CTC char mask derived from argmax(preds).  Everything
  else (loss, centers update) is O(n_class*feat) math done on host from the
  8 per-core partial sums:
    upd[c]      = ALPHA*(counts[c]*centers[c] - S[c]) / (1+counts[c])
    new_centers = centers - upd
    loss        = 0.5*(sum_c counts[c]*|centers[c]|^2 - 2*<centers,S> + q)

  On device per chunk of 128 batch rows (batch on partitions):
    m[b,t]   = reduce_max over classes            (DVE)
    eq       = (preds == m)  exact one-hot        (DVE)
    sh       = eq[t]*eq[t+1] shifted product      (GPSIMD)
    rep[b,t] = sum_c sh                           (DVE)  == 1 iff argmax repeats
    mask     = (1-eq[...,BLANK])*(1-rep)          (tiny)
    moh      = one-hot(labels + 85*(1-mask)) bf16 (DVE)  masked one-hot
    psum[85,129] += moh_t.T @ [x_t | x_t^2 | 1]   (PE, bf16 ops, fp32 accum)
"""

import sys

sys.path.insert(0, "/opt/trn_rl_repo")

from contextlib import ExitStack

import numpy as np

import concourse.bacc as bacc
import concourse.bass as bass
import concourse.mybir as mybir
import concourse.tile as tile
from concourse.bass_utils import run_bass_kernel_spmd

N_CLASS = 85
BLANK = 84
ALPHA = 0.05
B, T, D = 4096, 96, 64
NCORES = 8
BS = B // NCORES  # 512 batch rows per core
TH = T // 2  # half-chunk along t for DMA tiles

f32 = mybir.dt.float32
i32 = mybir.dt.int32
i16 = mybir.dt.int16
bf16 = mybir.dt.bfloat16

FDP = T * N_CLASS  # 8160 preds elems per row
NOUT = 2 * D + 1  # 129 output cols: S | SQ | counts


def build_program(bs: int = BS):
    nchunk = bs // 128
    nc = bacc.Bacc(
        "TRN2", target_bir_lowering=False, debug=False, num_devices=NCORES
    )
    preds = nc.dram_tensor("preds", [bs, T, N_CLASS], f32, kind="ExternalInput").ap()
    feats = nc.dram_tensor("features", [bs, T, D], f32, kind="ExternalInput").ap()
    labels = nc.dram_tensor("labels", [bs, T], i32, kind="ExternalInput").ap()
    out = nc.dram_tensor("out", [N_CLASS, NOUT], f32, kind="ExternalOutput").ap()

    CP = N_CLASS + 1  # 86: class dim padded so bf16 shifted views stay 4B-aligned

    with tile.TileContext(nc) as tc, ExitStack() as ctx:
        consts = ctx.enter_context(tc.tile_pool(name="consts", bufs=1))
        ppool = ctx.enter_context(tc.tile_pool(name="ppool", bufs=2))
        xpool = ctx.enter_context(tc.tile_pool(name="xpool", bufs=2))
        work = ctx.enter_context(tc.tile_pool(name="work", bufs=1))
        mmin = ctx.enter_context(tc.tile_pool(name="mmin", bufs=2))
        small = ctx.enter_context(tc.tile_pool(name="small", bufs=2))
        psum_pool = ctx.enter_context(
            tc.tile_pool(name="psum", bufs=1, space="PSUM")
        )

        # per-t scatter constants: tb85[p,t] = (t%16)*85, jp1[p,t] = (t%16)+1
        TB = 16  # t-block size for local_scatter (16*85=1360 elems < 2047)
        NB = T // TB
        tb85i = consts.tile([128, T], i16)
        nc.gpsimd.iota(
            tb85i[:], pattern=[[0, NB], [N_CLASS, TB]], base=0, channel_multiplier=0
        )
        jp1i = consts.tile([128, T], i16)
        nc.gpsimd.iota(
            jp1i[:], pattern=[[0, NB], [1, TB]], base=1, channel_multiplier=0
        )
        tb85f = consts.tile([128, T], f32)
        nc.scalar.copy(tb85f[:], tb85i[:])
        jp1f = consts.tile([128, T], f32)
        nc.scalar.copy(jp1f[:], jp1i[:])
        ones16 = consts.tile([128, TB], bf16)
        nc.vector.memset(ones16[:], 1.0)

        psumA = psum_pool.tile([N_CLASS, D], f32)  # S
        psumB = psum_pool.tile([N_CLASS, D + 1], f32)  # SQ | counts

        for c in range(nchunk):
            brange = slice(c * 128, (c + 1) * 128)

            # ---- load; dd[p,t,c] = preds - max <= 0, exactly 0 at argmax ----
            dd = work.tile([128, T, CP], bf16, tag="dd")
            ddf = dd[:].rearrange("p t c -> p (t c)")
            m = small.tile([128, T], f32, tag="m")
            mn = small.tile([128, T], f32, tag="mn")
            xb = mmin.tile([128, T, D], bf16, tag="xb")
            xsq = mmin.tile([128, T, D + 1], bf16, tag="xsq")

            nc.vector.memset(dd[:, :, N_CLASS], -1e30)
            for h in range(2):
                ts_ = slice(h * TH, (h + 1) * TH)
                ph = ppool.tile([128, TH, N_CLASS], f32, tag="ph")
                nc.gpsimd.dma_start(ph[:], preds[brange, ts_, :])
                xh = xpool.tile([128, TH, D], f32, tag="xh")
                nc.gpsimd.dma_start(xh[:], feats[brange, ts_, :])

                nc.vector.tensor_reduce(
                    m[:, ts_], ph[:], axis=mybir.AxisListType.X, op=mybir.AluOpType.max
                )
                nc.vector.tensor_scalar_mul(mn[:, ts_], m[:, ts_], -1.0)
                mb = mn[:, ts_].unsqueeze(2).broadcast_to([128, TH, N_CLASS])
                nc.gpsimd.tensor_tensor(
                    dd[:, ts_, 0:N_CLASS], ph[:], mb, op=mybir.AluOpType.add
                )
                # features: bf16 copy and squares (ScalarE)
                nc.scalar.copy(xb[:, ts_, :], xh[:])
                nc.scalar.square(xsq[:, ts_, 0:D], xh[:])

            nc.vector.memset(xsq[:, :, D], 1.0)

            labI = small.tile([128, T], i32, tag="labI")
            nc.gpsimd.dma_start(labI[:], labels[brange, :])

            # ---- repeated argmax: max_c(dd_t + dd_{t+1}) == 0 (exact) ----
            dsum = work.tile([128, (T - 1) * CP], bf16, tag="dsum")
            nc.vector.tensor_add(
                dsum[:], ddf[:, 0 : (T - 1) * CP], ddf[:, CP : T * CP]
            )
            zmax = small.tile([128, T], f32, tag="zmax")
            nc.vector.tensor_reduce(
                zmax[:, 0 : T - 1],
                dsum[:].rearrange("p (t c) -> p t c", c=CP),
                axis=mybir.AxisListType.X,
                op=mybir.AluOpType.max,
            )
            nc.vector.memset(zmax[:, T - 1 : T], -1.0)

            # ---- mask = (dd[..,BLANK] != 0) * (zmax != 0) ----
            inv84 = small.tile([128, T], f32, tag="inv84")
            nc.vector.tensor_single_scalar(
                inv84[:], dd[:, :, BLANK], 0.0, op=mybir.AluOpType.not_equal
            )
            invrep = small.tile([128, T], f32, tag="invrep")
            nc.vector.tensor_single_scalar(
                invrep[:], zmax[:], 0.0, op=mybir.AluOpType.not_equal
            )
            mask = small.tile([128, T], f32, tag="mask")
            nc.vector.tensor_mul(mask[:], invrep[:], inv84[:])

            # ---- scatter indices: idx = mask*((t%16)*85 + lab + jp1) - jp1 ----
            # masked-out slots get distinct negatives -(t%16+1) (ignored by
            # local_scatter); kept slots get (t%16)*85 + label.
            labf = small.tile([128, T], f32, tag="labf")
            nc.scalar.copy(labf[:], labI[:])
            s1 = small.tile([128, T], f32, tag="s1")
            nc.vector.tensor_add(s1[:], tb85f[:], labf[:])
            s2 = small.tile([128, T], f32, tag="s2")
            nc.vector.tensor_add(s2[:], s1[:], jp1f[:])
            s3 = small.tile([128, T], f32, tag="s3")
            nc.vector.tensor_mul(s3[:], s2[:], mask[:])
            idx16 = small.tile([128, T], i16, tag="idx16")
            nc.vector.tensor_sub(idx16[:], s3[:], jp1f[:])

            # ---- masked one-hot (bf16) via gpsimd local scatter ----
            moh = work.tile([128, FDP], bf16, tag="moh")
            moh3 = moh[:].rearrange("p (t c) -> p t c", c=N_CLASS)
            for blk in range(NB):
                nc.gpsimd.local_scatter(
                    moh[:, blk * TB * N_CLASS : (blk + 1) * TB * N_CLASS],
                    ones16[:],
                    idx16[:, blk * TB : (blk + 1) * TB],
                    channels=128,
                    num_elems=TB * N_CLASS,
                    num_idxs=TB,
                )

            # ---- segment sums via PE ----
            for t in range(T):
                first = c == 0 and t == 0
                last = c == nchunk - 1 and t == T - 1
                nc.tensor.matmul(
                    psumA[:],
                    moh3[:, t, :],
                    xb[:, t, :],
                    start=first,
                    stop=last,
                )
                nc.tensor.matmul(
                    psumB[:],
                    moh3[:, t, :],
                    xsq[:, t, :],
                    start=first,
                    stop=last,
                )

        outT = consts.tile([N_CLASS, NOUT], f32)
        nc.scalar.copy(outT[:, 0:D], psumA[:])
        nc.scalar.copy(outT[:, D : 2 * D + 1], psumB[:])
        nc.gpsimd.dma_start(out[:], outT[:])

    nc.compile()
    return nc


_prog_cache: dict[int, object] = {}


def _get_program(bs: int = BS):
    if bs not in _prog_cache:
        _prog_cache[bs] = build_program(bs)
    return _prog_cache[bs]


def combine_partials(parts: np.ndarray, centers: np.ndarray):
    """parts: [ncores, N_CLASS, NOUT] fp32 device partials -> (loss, new_centers)."""
    tot = parts.astype(np.float64).sum(axis=0)
    S = tot[:, 0:D]
    SQ = tot[:, D : 2 * D]
    counts = tot[:, 2 * D]
    q = SQ.sum()
    c64 = centers.astype(np.float64)
    upd = ALPHA * (counts[:, None] * c64 - S) / (1.0 + counts[:, None])
    new_centers = (c64 - upd).astype(np.float32)
    loss = 0.5 * ((counts * (c64**2).sum(1)).sum() - 2.0 * (c64 * S).sum() + q)
    return np.float32(loss), new_centers


def kernel(preds, features, labels, centers):
    nc = _get_program()
    in_maps = []
    for k in range(NCORES):
        sl = slice(k * BS, (k + 1) * BS)
        in_maps.append(
            {
                "preds": np.ascontiguousarray(preds[sl], dtype=np.float32),
                "features": np.ascontiguousarray(features[sl], dtype=np.float32),
                "labels": np.ascontiguousarray(labels[sl], dtype=np.int32),
            }
        )
    res = run_bass_kernel_spmd(nc, in_maps, core_ids=list(range(NCORES)))
    parts = np.stack([res.results[k]["out"] for k in range(NCORES)])
    return combine_partials(parts, np.asarray(centers, dtype=np.float32))


# revision 12
# speedup vs baseline: 1.2429x; 1.0816x over previous
"""CenterCTCLoss Trainium2 kernel.

Strategy (data-parallel over batch, 8 cores, 512 rows each):
  The reference computation collapses to three segment statistics per core:
    counts[c] = sum of mask where labels==c
    S[c,d]    = sum of mask*features where labels==c
    SQ[c,d]   = sum of mask*features^2 where labels==c   (q = SQ.sum())
  where mask is the CTC char mask derived from argmax(preds).  Everything
  else (loss, centers update) is O(n_class*feat) math done on host from the
  8 per-core partial sums:
    upd[c]      = ALPHA*(counts[c]*centers[c] - S[c]) / (1+counts[c])
    new_centers = centers - upd
    loss        = 0.5*(sum_c counts[c]*|centers[c]|^2 - 2*<centers,S> + q)

  On device per chunk of 128 batch rows (batch on partitions):
    m[b,t]   = reduce_max over classes            (DVE)
    eq       = (preds == m)  exact one-hot        (DVE)
    sh       = eq[t]*eq[t+1] shifted product      (GPSIMD)
    rep[b,t] = sum_c sh                           (DVE)  == 1 iff argmax repeats
    mask     = (1-eq[...,BLANK])*(1-rep)          (tiny)
    moh      = one-hot(labels + 85*(1-mask)) bf16 (DVE)  masked one-hot
    psum[85,129] += moh_t.T @ [x_t | x_t^2 | 1]   (PE, bf16 ops, fp32 accum)
"""

import sys

sys.path.insert(0, "/opt/trn_rl_repo")

from contextlib import ExitStack

import numpy as np

import concourse.bacc as bacc
import concourse.bass as bass
import concourse.mybir as mybir
import concourse.tile as tile
from concourse.bass_utils import run_bass_kernel_spmd

N_CLASS = 85
BLANK = 84
ALPHA = 0.05
B, T, D = 4096, 96, 64
NCORES = 8
BS = B // NCORES  # 512 batch rows per core
TH = T // 2  # half-chunk along t for DMA tiles

f32 = mybir.dt.float32
i32 = mybir.dt.int32
i16 = mybir.dt.int16
bf16 = mybir.dt.bfloat16

FDP = T * N_CLASS  # 8160 preds elems per row
NOUT = 2 * D + 1  # 129 output cols: S | SQ | counts


def build_program(bs: int = BS):
    nchunk = bs // 128
    nc = bacc.Bacc(
        "TRN2", target_bir_lowering=False, debug=False, num_devices=NCORES
    )
    preds = nc.dram_tensor("preds", [bs, T, N_CLASS], f32, kind="ExternalInput").ap()
    feats = nc.dram_tensor("features", [bs, T, D], f32, kind="ExternalInput").ap()
    labels = nc.dram_tensor("labels", [bs, T], i32, kind="ExternalInput").ap()
    out = nc.dram_tensor("out", [N_CLASS, NOUT], f32, kind="ExternalOutput").ap()

    CP = N_CLASS + 1  # 86: class dim padded so bf16 shifted views stay 4B-aligned

    with tile.TileContext(nc) as tc, ExitStack() as ctx:
        consts = ctx.enter_context(tc.tile_pool(name="consts", bufs=1))
        ppool = ctx.enter_context(tc.tile_pool(name="ppool", bufs=2))
        xpool = ctx.enter_context(tc.tile_pool(name="xpool", bufs=2))
        work = ctx.enter_context(tc.tile_pool(name="work", bufs=1))
        work2 = ctx.enter_context(tc.tile_pool(name="work2", bufs=2))
        mmin = ctx.enter_context(tc.tile_pool(name="mmin", bufs=2))
        small = ctx.enter_context(tc.tile_pool(name="small", bufs=2))
        psum_pool = ctx.enter_context(
            tc.tile_pool(name="psum", bufs=1, space="PSUM")
        )

        # per-t scatter constants: tb85[p,t] = (t%16)*85, jp1[p,t] = (t%16)+1
        TB = 16  # t-block size for local_scatter (16*85=1360 elems < 2047)
        NB = T // TB
        tb85i = consts.tile([128, T], i16)
        nc.gpsimd.iota(
            tb85i[:], pattern=[[0, NB], [N_CLASS, TB]], base=0, channel_multiplier=0
        )
        jp1i = consts.tile([128, T], i16)
        nc.gpsimd.iota(
            jp1i[:], pattern=[[0, NB], [1, TB]], base=1, channel_multiplier=0
        )
        tb85f = consts.tile([128, T], f32)
        nc.scalar.copy(tb85f[:], tb85i[:])
        jp1f = consts.tile([128, T], f32)
        nc.scalar.copy(jp1f[:], jp1i[:])
        ones16 = consts.tile([128, TB], bf16)
        nc.vector.memset(ones16[:], 1.0)

        psumA = psum_pool.tile([N_CLASS, D], f32)  # S
        psumB = psum_pool.tile([N_CLASS, D + 1], f32)  # SQ | counts

        def stage1(c):
            brange = slice(c * 128, (c + 1) * 128)
            # load; dd[p,t,c] = preds - max <= 0, exactly 0 at argmax
            dd = work2.tile([128, T, CP], bf16, tag="dd")
            m = small.tile([128, T], f32, tag="m")
            mn = small.tile([128, T], f32, tag="mn")
            xb = mmin.tile([128, T, D], bf16, tag="xb")
            xsq = mmin.tile([128, T, D + 1], bf16, tag="xsq")
            labI = small.tile([128, T], i32, tag="labI")
            nc.gpsimd.dma_start(labI[:], labels[brange, :])
            nc.vector.memset(dd[:, :, N_CLASS], -1e30)
            for h in range(2):
                ts_ = slice(h * TH, (h + 1) * TH)
                ph = ppool.tile([128, TH, N_CLASS], f32, tag="ph")
                nc.gpsimd.dma_start(ph[:], preds[brange, ts_, :])
                xh = xpool.tile([128, TH, D], f32, tag="xh")
                nc.gpsimd.dma_start(xh[:], feats[brange, ts_, :])

                nc.vector.tensor_reduce(
                    m[:, ts_], ph[:], axis=mybir.AxisListType.X, op=mybir.AluOpType.max
                )
                nc.vector.tensor_scalar_mul(mn[:, ts_], m[:, ts_], -1.0)
                mb = mn[:, ts_].unsqueeze(2).broadcast_to([128, TH, N_CLASS])
                nc.gpsimd.tensor_tensor(
                    dd[:, ts_, 0:N_CLASS], ph[:], mb, op=mybir.AluOpType.add
                )
                # features: bf16 copy and squares (ScalarE)
                nc.scalar.copy(xb[:, ts_, :], xh[:])
                nc.scalar.square(xsq[:, ts_, 0:D], xh[:])
            nc.vector.memset(xsq[:, :, D], 1.0)
            return dict(dd=dd, xb=xb, xsq=xsq, labI=labI)

        def stage2(c, st):
            dd, xb, xsq, labI = st["dd"], st["xb"], st["xsq"], st["labI"]
            ddf = dd[:].rearrange("p t c -> p (t c)")

            # repeated argmax: max_c(dd_t + dd_{t+1}) == 0 (exact)
            dsum = work.tile([128, (T - 1) * CP], bf16, tag="dsum")
            nc.vector.tensor_add(
                dsum[:], ddf[:, 0 : (T - 1) * CP], ddf[:, CP : T * CP]
            )
            zmax = small.tile([128, T], f32, tag="zmax")
            nc.vector.tensor_reduce(
                zmax[:, 0 : T - 1],
                dsum[:].rearrange("p (t c) -> p t c", c=CP),
                axis=mybir.AxisListType.X,
                op=mybir.AluOpType.max,
            )
            nc.vector.memset(zmax[:, T - 1 : T], -1.0)

            # mask = (dd[..,BLANK] != 0) * (zmax != 0)
            inv84 = small.tile([128, T], f32, tag="inv84")
            nc.vector.tensor_single_scalar(
                inv84[:], dd[:, :, BLANK], 0.0, op=mybir.AluOpType.not_equal
            )
            invrep = small.tile([128, T], f32, tag="invrep")
            nc.vector.tensor_single_scalar(
                invrep[:], zmax[:], 0.0, op=mybir.AluOpType.not_equal
            )
            mask = small.tile([128, T], f32, tag="mask")
            nc.vector.tensor_mul(mask[:], invrep[:], inv84[:])

            # scatter indices: idx = mask*((t%16)*85 + lab + jp1) - jp1
            # masked-out slots get distinct negatives -(t%16+1) (ignored by
            # local_scatter); kept slots get (t%16)*85 + label.
            labf = small.tile([128, T], f32, tag="labf")
            nc.scalar.copy(labf[:], labI[:])
            s1 = small.tile([128, T], f32, tag="s1")
            nc.vector.tensor_add(s1[:], tb85f[:], labf[:])
            s2 = small.tile([128, T], f32, tag="s2")
            nc.vector.tensor_add(s2[:], s1[:], jp1f[:])
            s3 = small.tile([128, T], f32, tag="s3")
            nc.vector.tensor_mul(s3[:], s2[:], mask[:])
            idx16 = small.tile([128, T], i16, tag="idx16")
            nc.vector.tensor_sub(idx16[:], s3[:], jp1f[:])

            # masked one-hot (bf16) via gpsimd local scatter
            moh = work.tile([128, FDP], bf16, tag="moh")
            moh3 = moh[:].rearrange("p (t c) -> p t c", c=N_CLASS)
            for blk in range(NB):
                nc.gpsimd.local_scatter(
                    moh[:, blk * TB * N_CLASS : (blk + 1) * TB * N_CLASS],
                    ones16[:],
                    idx16[:, blk * TB : (blk + 1) * TB],
                    channels=128,
                    num_elems=TB * N_CLASS,
                    num_idxs=TB,
                )

            # segment sums via PE
            for t in range(T):
                first = c == 0 and t == 0
                last = c == nchunk - 1 and t == T - 1
                nc.tensor.matmul(
                    psumA[:], moh3[:, t, :], xb[:, t, :], start=first, stop=last
                )
                nc.tensor.matmul(
                    psumB[:], moh3[:, t, :], xsq[:, t, :], start=first, stop=last
                )

        # two-stage software pipeline: stage1(c+1) is emitted before
        # stage2(c) so GPSIMD/DMA of the next chunk overlap the mask/
        # scatter/matmul tail of the current one.
        pending = None
        for c in range(nchunk + 1):
            if c < nchunk:
                st = stage1(c)
            if pending is not None:
                stage2(c - 1, pending)
            pending = st if c < nchunk else None

        outT = consts.tile([N_CLASS, NOUT], f32)
        nc.scalar.copy(outT[:, 0:D], psumA[:])
        nc.scalar.copy(outT[:, D : 2 * D + 1], psumB[:])
        nc.gpsimd.dma_start(out[:], outT[:])

    nc.compile()
    return nc


_prog_cache: dict[int, object] = {}


def _get_program(bs: int = BS):
    if bs not in _prog_cache:
        _prog_cache[bs] = build_program(bs)
    return _prog_cache[bs]


def combine_partials(parts: np.ndarray, centers: np.ndarray):
    """parts: [ncores, N_CLASS, NOUT] fp32 device partials -> (loss, new_centers)."""
    tot = parts.astype(np.float64).sum(axis=0)
    S = tot[:, 0:D]
    SQ = tot[:, D : 2 * D]
    counts = tot[:, 2 * D]
    q = SQ.sum()
    c64 = centers.astype(np.float64)
    upd = ALPHA * (counts[:, None] * c64 - S) / (1.0 + counts[:, None])
    new_centers = (c64 - upd).astype(np.float32)
    loss = 0.5 * ((counts * (c64**2).sum(1)).sum() - 2.0 * (c64 * S).sum() + q)
    return np.float32(loss), new_centers


def kernel(preds, features, labels, centers):
    nc = _get_program()
    in_maps = []
    for k in range(NCORES):
        sl = slice(k * BS, (k + 1) * BS)
        in_maps.append(
            {
                "preds": np.ascontiguousarray(preds[sl], dtype=np.float32),
                "features": np.ascontiguousarray(features[sl], dtype=np.float32),
                "labels": np.ascontiguousarray(labels[sl], dtype=np.int32),
            }
        )
    res = run_bass_kernel_spmd(nc, in_maps, core_ids=list(range(NCORES)))
    parts = np.stack([res.results[k]["out"] for k in range(NCORES)])
    return combine_partials(parts, np.asarray(centers, dtype=np.float32))


# revision 13
# speedup vs baseline: 1.2512x; 1.0067x over previous
"""CenterCTCLoss Trainium2 kernel.

Strategy (data-parallel over batch, 8 cores, 512 rows each):
  The reference computation collapses to three segment statistics per core:
    counts[c] = sum of mask where labels==c
    S[c,d]    = sum of mask*features where labels==c
    SQ[c,d]   = sum of mask*features^2 where labels==c   (q = SQ.sum())
  where mask is the CTC char mask derived from argmax(preds).  Everything
  else (loss, centers update) is O(n_class*feat) math done on host from the
  8 per-core partial sums:
    upd[c]      = ALPHA*(counts[c]*centers[c] - S[c]) / (1+counts[c])
    new_centers = centers - upd
    loss        = 0.5*(sum_c counts[c]*|centers[c]|^2 - 2*<centers,S> + q)

  On device per chunk of 128 batch rows (batch on partitions):
    m[b,t]   = reduce_max over classes            (DVE)
    eq       = (preds == m)  exact one-hot        (DVE)
    sh       = eq[t]*eq[t+1] shifted product      (GPSIMD)
    rep[b,t] = sum_c sh                           (DVE)  == 1 iff argmax repeats
    mask     = (1-eq[...,BLANK])*(1-rep)          (tiny)
    moh      = one-hot(labels + 85*(1-mask)) bf16 (DVE)  masked one-hot
    psum[85,129] += moh_t.T @ [x_t | x_t^2 | 1]   (PE, bf16 ops, fp32 accum)
"""

import sys

sys.path.insert(0, "/opt/trn_rl_repo")

from contextlib import ExitStack

import numpy as np

import concourse.bacc as bacc
import concourse.bass as bass
import concourse.mybir as mybir
import concourse.tile as tile
from concourse.bass_utils import run_bass_kernel_spmd

N_CLASS = 85
BLANK = 84
ALPHA = 0.05
B, T, D = 4096, 96, 64
NCORES = 8
BS = B // NCORES  # 512 batch rows per core
TH = T // 2  # half-chunk along t for DMA tiles

f32 = mybir.dt.float32
i32 = mybir.dt.int32
i16 = mybir.dt.int16
bf16 = mybir.dt.bfloat16

FDP = T * N_CLASS  # 8160 preds elems per row
NOUT = 2 * D + 1  # 129 output cols: S | SQ | counts


def build_program(bs: int = BS):
    nchunk = bs // 128
    nc = bacc.Bacc(
        "TRN2", target_bir_lowering=False, debug=False, num_devices=NCORES
    )
    preds = nc.dram_tensor("preds", [bs, T, N_CLASS], f32, kind="ExternalInput").ap()
    feats = nc.dram_tensor("features", [bs, T, D], f32, kind="ExternalInput").ap()
    labels = nc.dram_tensor("labels", [bs, T], i32, kind="ExternalInput").ap()
    out = nc.dram_tensor("out", [N_CLASS, NOUT], f32, kind="ExternalOutput").ap()

    CP = N_CLASS + 1  # 86: class dim padded so bf16 shifted views stay 4B-aligned

    with tile.TileContext(nc) as tc, ExitStack() as ctx:
        consts = ctx.enter_context(tc.tile_pool(name="consts", bufs=1))
        ppool = ctx.enter_context(tc.tile_pool(name="ppool", bufs=2))
        xpool = ctx.enter_context(tc.tile_pool(name="xpool", bufs=2))
        work = ctx.enter_context(tc.tile_pool(name="work", bufs=1))
        work2 = ctx.enter_context(tc.tile_pool(name="work2", bufs=2))
        mmin = ctx.enter_context(tc.tile_pool(name="mmin", bufs=2))
        small = ctx.enter_context(tc.tile_pool(name="small", bufs=2))
        psum_pool = ctx.enter_context(
            tc.tile_pool(name="psum", bufs=1, space="PSUM")
        )

        # per-t scatter constants: tb85[p,t] = (t%16)*85, jp1[p,t] = (t%16)+1
        TB = 16  # t-block size for local_scatter (16*85=1360 elems < 2047)
        NB = T // TB
        tb85i = consts.tile([128, T], i16)
        nc.gpsimd.iota(
            tb85i[:], pattern=[[0, NB], [N_CLASS, TB]], base=0, channel_multiplier=0
        )
        jp1i = consts.tile([128, T], i16)
        nc.gpsimd.iota(
            jp1i[:], pattern=[[0, NB], [1, TB]], base=1, channel_multiplier=0
        )
        tb85f = consts.tile([128, T], f32)
        nc.scalar.copy(tb85f[:], tb85i[:])
        jp1f = consts.tile([128, T], f32)
        nc.scalar.copy(jp1f[:], jp1i[:])
        ones16 = consts.tile([128, TB], bf16)
        nc.vector.memset(ones16[:], 1.0)

        psumA = psum_pool.tile([N_CLASS, D], f32)  # S
        psumB = psum_pool.tile([N_CLASS, D + 1], f32)  # SQ | counts

        def stage1(c):
            brange = slice(c * 128, (c + 1) * 128)
            # load; dd[p,t,c] = preds - max <= 0, exactly 0 at argmax
            dd = work2.tile([128, T, CP], bf16, tag="dd")
            m = small.tile([128, T], f32, tag="m")
            mn = small.tile([128, T], f32, tag="mn")
            xb = mmin.tile([128, T, D], bf16, tag="xb")
            xsq = mmin.tile([128, T, D + 1], bf16, tag="xsq")
            labI = small.tile([128, T], i32, tag="labI")
            nc.sync.dma_start(labI[:], labels[brange, :])
            nc.vector.memset(dd[:, :, N_CLASS], -1e30)
            for h in range(2):
                ts_ = slice(h * TH, (h + 1) * TH)
                ph = ppool.tile([128, TH, N_CLASS], f32, tag="ph")
                nc.sync.dma_start(ph[:], preds[brange, ts_, :])
                xh = xpool.tile([128, TH, D], f32, tag="xh")
                nc.scalar.dma_start(xh[:], feats[brange, ts_, :])

                nc.vector.tensor_reduce(
                    m[:, ts_], ph[:], axis=mybir.AxisListType.X, op=mybir.AluOpType.max
                )
                nc.vector.tensor_scalar_mul(mn[:, ts_], m[:, ts_], -1.0)
                mb = mn[:, ts_].unsqueeze(2).broadcast_to([128, TH, N_CLASS])
                nc.gpsimd.tensor_tensor(
                    dd[:, ts_, 0:N_CLASS], ph[:], mb, op=mybir.AluOpType.add
                )
                # features: bf16 copy and squares (ScalarE)
                nc.scalar.copy(xb[:, ts_, :], xh[:])
                nc.scalar.square(xsq[:, ts_, 0:D], xh[:])
            nc.vector.memset(xsq[:, :, D], 1.0)
            return dict(dd=dd, xb=xb, xsq=xsq, labI=labI)

        def stage2(c, st):
            dd, xb, xsq, labI = st["dd"], st["xb"], st["xsq"], st["labI"]
            ddf = dd[:].rearrange("p t c -> p (t c)")

            # repeated argmax: max_c(dd_t + dd_{t+1}) == 0 (exact)
            dsum = work.tile([128, (T - 1) * CP], bf16, tag="dsum")
            nc.vector.tensor_add(
                dsum[:], ddf[:, 0 : (T - 1) * CP], ddf[:, CP : T * CP]
            )
            zmax = small.tile([128, T], f32, tag="zmax")
            nc.vector.tensor_reduce(
                zmax[:, 0 : T - 1],
                dsum[:].rearrange("p (t c) -> p t c", c=CP),
                axis=mybir.AxisListType.X,
                op=mybir.AluOpType.max,
            )
            nc.vector.memset(zmax[:, T - 1 : T], -1.0)

            # mask = (dd[..,BLANK] != 0) * (zmax != 0)
            inv84 = small.tile([128, T], f32, tag="inv84")
            nc.vector.tensor_single_scalar(
                inv84[:], dd[:, :, BLANK], 0.0, op=mybir.AluOpType.not_equal
            )
            invrep = small.tile([128, T], f32, tag="invrep")
            nc.vector.tensor_single_scalar(
                invrep[:], zmax[:], 0.0, op=mybir.AluOpType.not_equal
            )
            mask = small.tile([128, T], f32, tag="mask")
            nc.vector.tensor_mul(mask[:], invrep[:], inv84[:])

            # scatter indices: idx = mask*((t%16)*85 + lab + jp1) - jp1
            # masked-out slots get distinct negatives -(t%16+1) (ignored by
            # local_scatter); kept slots get (t%16)*85 + label.
            labf = small.tile([128, T], f32, tag="labf")
            nc.scalar.copy(labf[:], labI[:])
            s1 = small.tile([128, T], f32, tag="s1")
            nc.vector.tensor_add(s1[:], tb85f[:], labf[:])
            s2 = small.tile([128, T], f32, tag="s2")
            nc.vector.tensor_add(s2[:], s1[:], jp1f[:])
            s3 = small.tile([128, T], f32, tag="s3")
            nc.vector.tensor_mul(s3[:], s2[:], mask[:])
            idx16 = small.tile([128, T], i16, tag="idx16")
            nc.vector.tensor_sub(idx16[:], s3[:], jp1f[:])

            # masked one-hot (bf16) via gpsimd local scatter
            moh = work.tile([128, FDP], bf16, tag="moh")
            moh3 = moh[:].rearrange("p (t c) -> p t c", c=N_CLASS)
            for blk in range(NB):
                nc.gpsimd.local_scatter(
                    moh[:, blk * TB * N_CLASS : (blk + 1) * TB * N_CLASS],
                    ones16[:],
                    idx16[:, blk * TB : (blk + 1) * TB],
                    channels=128,
                    num_elems=TB * N_CLASS,
                    num_idxs=TB,
                )

            # segment sums via PE
            for t in range(T):
                first = c == 0 and t == 0
                last = c == nchunk - 1 and t == T - 1
                nc.tensor.matmul(
                    psumA[:], moh3[:, t, :], xb[:, t, :], start=first, stop=last
                )
                nc.tensor.matmul(
                    psumB[:], moh3[:, t, :], xsq[:, t, :], start=first, stop=last
                )

        # two-stage software pipeline: stage1(c+1) is emitted before
        # stage2(c) so GPSIMD/DMA of the next chunk overlap the mask/
        # scatter/matmul tail of the current one.
        pending = None
        for c in range(nchunk + 1):
            if c < nchunk:
                st = stage1(c)
            if pending is not None:
                stage2(c - 1, pending)
            pending = st if c < nchunk else None

        outT = consts.tile([N_CLASS, NOUT], f32)
        nc.scalar.copy(outT[:, 0:D], psumA[:])
        nc.scalar.copy(outT[:, D : 2 * D + 1], psumB[:])
        nc.sync.dma_start(out[:], outT[:])

    nc.compile()
    return nc


_prog_cache: dict[int, object] = {}


def _get_program(bs: int = BS):
    if bs not in _prog_cache:
        _prog_cache[bs] = build_program(bs)
    return _prog_cache[bs]


def combine_partials(parts: np.ndarray, centers: np.ndarray):
    """parts: [ncores, N_CLASS, NOUT] fp32 device partials -> (loss, new_centers)."""
    tot = parts.astype(np.float64).sum(axis=0)
    S = tot[:, 0:D]
    SQ = tot[:, D : 2 * D]
    counts = tot[:, 2 * D]
    q = SQ.sum()
    c64 = centers.astype(np.float64)
    upd = ALPHA * (counts[:, None] * c64 - S) / (1.0 + counts[:, None])
    new_centers = (c64 - upd).astype(np.float32)
    loss = 0.5 * ((counts * (c64**2).sum(1)).sum() - 2.0 * (c64 * S).sum() + q)
    return np.float32(loss), new_centers


def kernel(preds, features, labels, centers):
    nc = _get_program()
    in_maps = []
    for k in range(NCORES):
        sl = slice(k * BS, (k + 1) * BS)
        in_maps.append(
            {
                "preds": np.ascontiguousarray(preds[sl], dtype=np.float32),
                "features": np.ascontiguousarray(features[sl], dtype=np.float32),
                "labels": np.ascontiguousarray(labels[sl], dtype=np.int32),
            }
        )
    res = run_bass_kernel_spmd(nc, in_maps, core_ids=list(range(NCORES)))
    parts = np.stack([res.results[k]["out"] for k in range(NCORES)])
    return combine_partials(parts, np.asarray(centers, dtype=np.float32))


# revision 14
# speedup vs baseline: 1.2527x; 1.0011x over previous
"""CenterCTCLoss Trainium2 kernel.

Strategy (data-parallel over batch, 8 cores, 512 rows each):
  The reference computation collapses to three segment statistics per core:
    counts[c] = sum of mask where labels==c
    S[c,d]    = sum of mask*features where labels==c
    SQ[c,d]   = sum of mask*features^2 where labels==c   (q = SQ.sum())
  where mask is the CTC char mask derived from argmax(preds).  Everything
  else (loss, centers update) is O(n_class*feat) math done on host from the
  8 per-core partial sums:
    upd[c]      = ALPHA*(counts[c]*centers[c] - S[c]) / (1+counts[c])
    new_centers = centers - upd
    loss        = 0.5*(sum_c counts[c]*|centers[c]|^2 - 2*<centers,S> + q)

  On device per chunk of 128 batch rows (batch on partitions):
    m[b,t]   = reduce_max over classes            (DVE)
    eq       = (preds == m)  exact one-hot        (DVE)
    sh       = eq[t]*eq[t+1] shifted product      (GPSIMD)
    rep[b,t] = sum_c sh                           (DVE)  == 1 iff argmax repeats
    mask     = (1-eq[...,BLANK])*(1-rep)          (tiny)
    moh      = one-hot(labels + 85*(1-mask)) bf16 (DVE)  masked one-hot
    psum[85,129] += moh_t.T @ [x_t | x_t^2 | 1]   (PE, bf16 ops, fp32 accum)
"""

import sys

sys.path.insert(0, "/opt/trn_rl_repo")

from contextlib import ExitStack

import numpy as np

import concourse.bacc as bacc
import concourse.bass as bass
import concourse.mybir as mybir
import concourse.tile as tile
from concourse.bass_utils import run_bass_kernel_spmd

N_CLASS = 85
BLANK = 84
ALPHA = 0.05
B, T, D = 4096, 96, 64
NCORES = 8
BS = B // NCORES  # 512 batch rows per core
TH = T // 2  # half-chunk along t for DMA tiles

f32 = mybir.dt.float32
i32 = mybir.dt.int32
i16 = mybir.dt.int16
bf16 = mybir.dt.bfloat16

FDP = T * N_CLASS  # 8160 preds elems per row
NOUT = 2 * D + 1  # 129 output cols: S | SQ | counts


def build_program(bs: int = BS):
    nchunk = bs // 128
    nc = bacc.Bacc(
        "TRN2", target_bir_lowering=False, debug=False, num_devices=NCORES
    )
    preds = nc.dram_tensor("preds", [bs, T, N_CLASS], f32, kind="ExternalInput").ap()
    feats = nc.dram_tensor("features", [bs, T, D], f32, kind="ExternalInput").ap()
    labels = nc.dram_tensor("labels", [bs, T], i32, kind="ExternalInput").ap()
    out = nc.dram_tensor("out", [N_CLASS, NOUT], f32, kind="ExternalOutput").ap()

    CP = N_CLASS + 1  # 86: class dim padded so bf16 shifted views stay 4B-aligned

    with tile.TileContext(nc) as tc, ExitStack() as ctx:
        consts = ctx.enter_context(tc.tile_pool(name="consts", bufs=1))
        ppool = ctx.enter_context(tc.tile_pool(name="ppool", bufs=2))
        xpool = ctx.enter_context(tc.tile_pool(name="xpool", bufs=2))
        work = ctx.enter_context(tc.tile_pool(name="work", bufs=1))
        work2 = ctx.enter_context(tc.tile_pool(name="work2", bufs=2))
        mmin = ctx.enter_context(tc.tile_pool(name="mmin", bufs=2))
        small = ctx.enter_context(tc.tile_pool(name="small", bufs=2))
        psum_pool = ctx.enter_context(
            tc.tile_pool(name="psum", bufs=1, space="PSUM")
        )

        # per-t scatter constants: tb85[p,t] = (t%16)*85, jp1[p,t] = (t%16)+1
        TB = 16  # t-block size for local_scatter (16*85=1360 elems < 2047)
        NB = T // TB
        tb85i = consts.tile([128, T], i16)
        nc.gpsimd.iota(
            tb85i[:], pattern=[[0, NB], [N_CLASS, TB]], base=0, channel_multiplier=0
        )
        jp1i = consts.tile([128, T], i16)
        nc.gpsimd.iota(
            jp1i[:], pattern=[[0, NB], [1, TB]], base=1, channel_multiplier=0
        )
        tb85f = consts.tile([128, T], f32)
        nc.scalar.copy(tb85f[:], tb85i[:])
        jp1f = consts.tile([128, T], f32)
        nc.scalar.copy(jp1f[:], jp1i[:])
        ones16 = consts.tile([128, TB], bf16)
        nc.vector.memset(ones16[:], 1.0)

        psumA = psum_pool.tile([N_CLASS, D], f32)  # S
        psumB = psum_pool.tile([N_CLASS, D + 1], f32)  # SQ | counts

        def stage1(c):
            brange = slice(c * 128, (c + 1) * 128)
            # load; dd[p,t,c] = preds - max <= 0, exactly 0 at argmax
            dd = work2.tile([128, T, CP], bf16, tag="dd")
            m = small.tile([128, T], f32, tag="m")
            mn = small.tile([128, T], f32, tag="mn")
            xb = mmin.tile([128, T, D], bf16, tag="xb")
            xsq = mmin.tile([128, T, D + 1], bf16, tag="xsq")
            labI = small.tile([128, T], i32, tag="labI")
            nc.sync.dma_start(labI[:], labels[brange, :])
            nc.vector.memset(dd[:, :, N_CLASS], -1e30)
            TQ = T // 4
            for h in range(4):
                ts_ = slice(h * TQ, (h + 1) * TQ)
                ph = ppool.tile([128, TQ, N_CLASS], f32, tag="ph")
                nc.sync.dma_start(ph[:], preds[brange, ts_, :])

                nc.vector.tensor_reduce(
                    m[:, ts_], ph[:], axis=mybir.AxisListType.X, op=mybir.AluOpType.max
                )
                nc.vector.tensor_scalar_mul(mn[:, ts_], m[:, ts_], -1.0)
                mb = mn[:, ts_].unsqueeze(2).broadcast_to([128, TQ, N_CLASS])
                nc.gpsimd.tensor_tensor(
                    dd[:, ts_, 0:N_CLASS], ph[:], mb, op=mybir.AluOpType.add
                )
            for h in range(2):
                ts_ = slice(h * TH, (h + 1) * TH)
                xh = xpool.tile([128, TH, D], f32, tag="xh")
                nc.scalar.dma_start(xh[:], feats[brange, ts_, :])
                # features: bf16 copy and squares (ScalarE)
                nc.scalar.copy(xb[:, ts_, :], xh[:])
                nc.scalar.square(xsq[:, ts_, 0:D], xh[:])
            nc.vector.memset(xsq[:, :, D], 1.0)
            # blank-argmax indicator and scatter indices only need dd/labels
            inv84 = small.tile([128, T], f32, tag="inv84")
            nc.vector.tensor_single_scalar(
                inv84[:], dd[:, :, BLANK], 0.0, op=mybir.AluOpType.not_equal
            )
            labf = small.tile([128, T], f32, tag="labf")
            nc.scalar.copy(labf[:], labI[:])
            idx16 = small.tile([128, T], i16, tag="idx16")
            nc.vector.tensor_add(idx16[:], tb85f[:], labf[:])
            return dict(dd=dd, xb=xb, xsq=xsq, inv84=inv84, idx16=idx16)

        def stage2(c, st):
            dd, xb, xsq = st["dd"], st["xb"], st["xsq"]
            inv84, idx16 = st["inv84"], st["idx16"]
            ddf = dd[:].rearrange("p t c -> p (t c)")

            # repeated argmax: max_c(dd_t + dd_{t+1}) == 0 (exact)
            dsum = work.tile([128, (T - 1) * CP], bf16, tag="dsum")
            nc.vector.tensor_add(
                dsum[:], ddf[:, 0 : (T - 1) * CP], ddf[:, CP : T * CP]
            )
            zmax = small.tile([128, T], f32, tag="zmax")
            nc.vector.tensor_reduce(
                zmax[:, 0 : T - 1],
                dsum[:].rearrange("p (t c) -> p t c", c=CP),
                axis=mybir.AxisListType.X,
                op=mybir.AluOpType.max,
            )
            nc.vector.memset(zmax[:, T - 1 : T], -1.0)

            # mask = (dd[..,BLANK] != 0) * (zmax != 0), scattered as DATA:
            # moh[p, (t%16)*85+lab] = mask[p,t] (indices never masked, always
            # distinct within a 16-t block)
            invrep = small.tile([128, T], f32, tag="invrep")
            nc.vector.tensor_single_scalar(
                invrep[:], zmax[:], 0.0, op=mybir.AluOpType.not_equal
            )
            maskb = small.tile([128, T], bf16, tag="maskb")
            nc.vector.tensor_mul(maskb[:], invrep[:], inv84[:])

            # masked one-hot (bf16) via gpsimd local scatter
            moh = work.tile([128, FDP], bf16, tag="moh")
            moh3 = moh[:].rearrange("p (t c) -> p t c", c=N_CLASS)
            for blk in range(NB):
                nc.gpsimd.local_scatter(
                    moh[:, blk * TB * N_CLASS : (blk + 1) * TB * N_CLASS],
                    maskb[:, blk * TB : (blk + 1) * TB],
                    idx16[:, blk * TB : (blk + 1) * TB],
                    channels=128,
                    num_elems=TB * N_CLASS,
                    num_idxs=TB,
                )

            # segment sums via PE
            for t in range(T):
                first = c == 0 and t == 0
                last = c == nchunk - 1 and t == T - 1
                nc.tensor.matmul(
                    psumA[:], moh3[:, t, :], xb[:, t, :], start=first, stop=last
                )
                nc.tensor.matmul(
                    psumB[:], moh3[:, t, :], xsq[:, t, :], start=first, stop=last
                )

        # two-stage software pipeline: stage1(c+1) is emitted before
        # stage2(c) so GPSIMD/DMA of the next chunk overlap the mask/
        # scatter/matmul tail of the current one.
        pending = None
        for c in range(nchunk + 1):
            if c < nchunk:
                st = stage1(c)
            if pending is not None:
                stage2(c - 1, pending)
            pending = st if c < nchunk else None

        outT = consts.tile([N_CLASS, NOUT], f32)
        nc.scalar.copy(outT[:, 0:D], psumA[:])
        nc.scalar.copy(outT[:, D : 2 * D + 1], psumB[:])
        nc.sync.dma_start(out[:], outT[:])

    nc.compile()
    return nc


_prog_cache: dict[int, object] = {}


def _get_program(bs: int = BS):
    if bs not in _prog_cache:
        _prog_cache[bs] = build_program(bs)
    return _prog_cache[bs]


def combine_partials(parts: np.ndarray, centers: np.ndarray):
    """parts: [ncores, N_CLASS, NOUT] fp32 device partials -> (loss, new_centers)."""
    tot = parts.astype(np.float64).sum(axis=0)
    S = tot[:, 0:D]
    SQ = tot[:, D : 2 * D]
    counts = tot[:, 2 * D]
    q = SQ.sum()
    c64 = centers.astype(np.float64)
    upd = ALPHA * (counts[:, None] * c64 - S) / (1.0 + counts[:, None])
    new_centers = (c64 - upd).astype(np.float32)
    loss = 0.5 * ((counts * (c64**2).sum(1)).sum() - 2.0 * (c64 * S).sum() + q)
    return np.float32(loss), new_centers


def kernel(preds, features, labels, centers):
    nc = _get_program()
    in_maps = []
    for k in range(NCORES):
        sl = slice(k * BS, (k + 1) * BS)
        in_maps.append(
            {
                "preds": np.ascontiguousarray(preds[sl], dtype=np.float32),
                "features": np.ascontiguousarray(features[sl], dtype=np.float32),
                "labels": np.ascontiguousarray(labels[sl], dtype=np.int32),
            }
        )
    res = run_bass_kernel_spmd(nc, in_maps, core_ids=list(range(NCORES)))
    parts = np.stack([res.results[k]["out"] for k in range(NCORES)])
    return combine_partials(parts, np.asarray(centers, dtype=np.float32))


# revision 15
# speedup vs baseline: 1.3095x; 1.0454x over previous
"""CenterCTCLoss Trainium2 kernel.

Strategy (data-parallel over batch, 8 cores, 512 rows each):
  The reference computation collapses to three segment statistics per core:
    counts[c] = sum of mask where labels==c
    S[c,d]    = sum of mask*features where labels==c
    SQ[c,d]   = sum of mask*features^2 where labels==c   (q = SQ.sum())
  where mask is the CTC char mask derived from argmax(preds).  Everything
  else (loss, centers update) is O(n_class*feat) math done on host from the
  8 per-core partial sums:
    upd[c]      = ALPHA*(counts[c]*centers[c] - S[c]) / (1+counts[c])
    new_centers = centers - upd
    loss        = 0.5*(sum_c counts[c]*|centers[c]|^2 - 2*<centers,S> + q)

  On device per chunk of 128 batch rows (batch on partitions):
    m[b,t]   = reduce_max over classes            (DVE)
    eq       = (preds == m)  exact one-hot        (DVE)
    sh       = eq[t]*eq[t+1] shifted product      (GPSIMD)
    rep[b,t] = sum_c sh                           (DVE)  == 1 iff argmax repeats
    mask     = (1-eq[...,BLANK])*(1-rep)          (tiny)
    moh      = one-hot(labels + 85*(1-mask)) bf16 (DVE)  masked one-hot
    psum[85,129] += moh_t.T @ [x_t | x_t^2 | 1]   (PE, bf16 ops, fp32 accum)
"""

import sys

sys.path.insert(0, "/opt/trn_rl_repo")

from contextlib import ExitStack

import numpy as np

import concourse.bacc as bacc
import concourse.bass as bass
import concourse.mybir as mybir
import concourse.tile as tile
from concourse.bass_utils import run_bass_kernel_spmd

N_CLASS = 85
BLANK = 84
ALPHA = 0.05
B, T, D = 4096, 96, 64
NCORES = 8
BS = B // NCORES  # 512 batch rows per core
TH = T // 2  # half-chunk along t for DMA tiles

f32 = mybir.dt.float32
i32 = mybir.dt.int32
i16 = mybir.dt.int16
bf16 = mybir.dt.bfloat16

FDP = T * N_CLASS  # 8160 preds elems per row
NOUT = 2 * D + 1  # 129 output cols: S | SQ | counts


def build_program(bs: int = BS):
    nchunk = bs // 128
    nc = bacc.Bacc(
        "TRN2", target_bir_lowering=False, debug=False, num_devices=NCORES
    )
    preds = nc.dram_tensor("preds", [bs, T, N_CLASS], f32, kind="ExternalInput").ap()
    feats = nc.dram_tensor("features", [bs, T, D], f32, kind="ExternalInput").ap()
    labels = nc.dram_tensor("labels", [bs, T], i32, kind="ExternalInput").ap()
    out = nc.dram_tensor("out", [N_CLASS, NOUT], f32, kind="ExternalOutput").ap()

    CP = N_CLASS + 1  # 86: class dim padded so bf16 shifted views stay 4B-aligned

    with tile.TileContext(nc) as tc, ExitStack() as ctx:
        consts = ctx.enter_context(tc.tile_pool(name="consts", bufs=1))
        ppool = ctx.enter_context(tc.tile_pool(name="ppool", bufs=4))
        xpool = ctx.enter_context(tc.tile_pool(name="xpool", bufs=2))
        work = ctx.enter_context(tc.tile_pool(name="work", bufs=1))
        work2 = ctx.enter_context(tc.tile_pool(name="work2", bufs=2))
        mmin = ctx.enter_context(tc.tile_pool(name="mmin", bufs=2))
        small = ctx.enter_context(tc.tile_pool(name="small", bufs=4))
        psum_pool = ctx.enter_context(
            tc.tile_pool(name="psum", bufs=1, space="PSUM")
        )

        # per-t scatter constants: tb85[p,t] = (t%16)*85, jp1[p,t] = (t%16)+1
        TB = 16  # t-block size for local_scatter (16*85=1360 elems < 2047)
        NB = T // TB
        tb85i = consts.tile([128, T], i16)
        nc.gpsimd.iota(
            tb85i[:], pattern=[[0, NB], [N_CLASS, TB]], base=0, channel_multiplier=0
        )
        jp1i = consts.tile([128, T], i16)
        nc.gpsimd.iota(
            jp1i[:], pattern=[[0, NB], [1, TB]], base=1, channel_multiplier=0
        )
        tb85f = consts.tile([128, T], f32)
        nc.scalar.copy(tb85f[:], tb85i[:])
        jp1f = consts.tile([128, T], f32)
        nc.scalar.copy(jp1f[:], jp1i[:])
        ones16 = consts.tile([128, TB], bf16)
        nc.vector.memset(ones16[:], 1.0)

        psumA = psum_pool.tile([N_CLASS, D], f32)  # S
        psumB = psum_pool.tile([N_CLASS, D + 1], f32)  # SQ | counts

        def stage1(c):
            brange = slice(c * 128, (c + 1) * 128)
            # load; dd[p,t,c] = preds - max <= 0, exactly 0 at argmax
            dd = work2.tile([128, T, CP], bf16, tag="dd")
            m = small.tile([128, T], f32, tag="m")
            xb = mmin.tile([128, T, D], bf16, tag="xb")
            xsq = mmin.tile([128, T, D + 1], bf16, tag="xsq")
            labI = small.tile([128, T], i32, tag="labI")
            nc.sync.dma_start(labI[:], labels[brange, :])
            nc.gpsimd.memset(dd[:, :, N_CLASS], -1e30)
            TQ = T // 4
            for h in range(4):
                ts_ = slice(h * TQ, (h + 1) * TQ)
                ph = ppool.tile([128, TQ, N_CLASS], f32, tag="ph")
                nc.sync.dma_start(ph[:], preds[brange, ts_, :])

                nc.vector.tensor_reduce(
                    m[:, ts_], ph[:], axis=mybir.AxisListType.X, op=mybir.AluOpType.max
                )
                mb = m[:, ts_].unsqueeze(2).broadcast_to([128, TQ, N_CLASS])
                nc.gpsimd.tensor_tensor(
                    dd[:, ts_, 0:N_CLASS], ph[:], mb, op=mybir.AluOpType.subtract
                )
            for h in range(2):
                ts_ = slice(h * TH, (h + 1) * TH)
                xh = xpool.tile([128, TH, D], f32, tag="xh")
                nc.scalar.dma_start(xh[:], feats[brange, ts_, :])
                # features: bf16 copy and squares (ScalarE)
                nc.scalar.copy(xb[:, ts_, :], xh[:])
                nc.scalar.square(xsq[:, ts_, 0:D], xh[:])
            nc.vector.memset(xsq[:, :, D], 1.0)
            # blank-argmax indicator and scatter indices only need dd/labels
            inv84 = small.tile([128, T], f32, tag="inv84")
            nc.vector.tensor_single_scalar(
                inv84[:], dd[:, :, BLANK], 0.0, op=mybir.AluOpType.not_equal
            )
            labf = small.tile([128, T], f32, tag="labf")
            nc.scalar.copy(labf[:], labI[:])
            idx16 = small.tile([128, T], i16, tag="idx16")
            nc.vector.tensor_add(idx16[:], tb85f[:], labf[:])
            return dict(dd=dd, xb=xb, xsq=xsq, inv84=inv84, idx16=idx16)

        def stage2(c, st):
            dd, xb, xsq = st["dd"], st["xb"], st["xsq"]
            inv84, idx16 = st["inv84"], st["idx16"]
            ddf = dd[:].rearrange("p t c -> p (t c)")

            # repeated argmax: max_c(dd_t + dd_{t+1}) == 0 (exact)
            dsum = work.tile([128, (T - 1) * CP], bf16, tag="dsum")
            nc.vector.tensor_add(
                dsum[:], ddf[:, 0 : (T - 1) * CP], ddf[:, CP : T * CP]
            )
            zmax = small.tile([128, T], f32, tag="zmax")
            nc.vector.tensor_reduce(
                zmax[:, 0 : T - 1],
                dsum[:].rearrange("p (t c) -> p t c", c=CP),
                axis=mybir.AxisListType.X,
                op=mybir.AluOpType.max,
            )
            nc.vector.memset(zmax[:, T - 1 : T], -1.0)

            # mask = (dd[..,BLANK] != 0) * (zmax != 0), scattered as DATA:
            # moh[p, (t%16)*85+lab] = mask[p,t] (indices never masked, always
            # distinct within a 16-t block)
            invrep = small.tile([128, T], f32, tag="invrep")
            nc.vector.tensor_single_scalar(
                invrep[:], zmax[:], 0.0, op=mybir.AluOpType.not_equal
            )
            maskb = small.tile([128, T], bf16, tag="maskb")
            nc.vector.tensor_mul(maskb[:], invrep[:], inv84[:])

            # masked one-hot (bf16) via gpsimd local scatter
            moh = work.tile([128, FDP], bf16, tag="moh")
            moh3 = moh[:].rearrange("p (t c) -> p t c", c=N_CLASS)
            for blk in range(NB):
                nc.gpsimd.local_scatter(
                    moh[:, blk * TB * N_CLASS : (blk + 1) * TB * N_CLASS],
                    maskb[:, blk * TB : (blk + 1) * TB],
                    idx16[:, blk * TB : (blk + 1) * TB],
                    channels=128,
                    num_elems=TB * N_CLASS,
                    num_idxs=TB,
                )

            # segment sums via PE
            for t in range(T):
                first = c == 0 and t == 0
                last = c == nchunk - 1 and t == T - 1
                nc.tensor.matmul(
                    psumA[:], moh3[:, t, :], xb[:, t, :], start=first, stop=last
                )
                nc.tensor.matmul(
                    psumB[:], moh3[:, t, :], xsq[:, t, :], start=first, stop=last
                )

        # two-stage software pipeline: stage1(c+1) is emitted before
        # stage2(c) so GPSIMD/DMA of the next chunk overlap the mask/
        # scatter/matmul tail of the current one.
        pending = None
        for c in range(nchunk + 1):
            if c < nchunk:
                st = stage1(c)
            if pending is not None:
                stage2(c - 1, pending)
            pending = st if c < nchunk else None

        outT = consts.tile([N_CLASS, NOUT], f32)
        nc.scalar.copy(outT[:, 0:D], psumA[:])
        nc.scalar.copy(outT[:, D : 2 * D + 1], psumB[:])
        nc.sync.dma_start(out[:], outT[:])

    nc.compile()
    return nc


_prog_cache: dict[int, object] = {}


def _get_program(bs: int = BS):
    if bs not in _prog_cache:
        _prog_cache[bs] = build_program(bs)
    return _prog_cache[bs]


def combine_partials(parts: np.ndarray, centers: np.ndarray):
    """parts: [ncores, N_CLASS, NOUT] fp32 device partials -> (loss, new_centers)."""
    tot = parts.astype(np.float64).sum(axis=0)
    S = tot[:, 0:D]
    SQ = tot[:, D : 2 * D]
    counts = tot[:, 2 * D]
    q = SQ.sum()
    c64 = centers.astype(np.float64)
    upd = ALPHA * (counts[:, None] * c64 - S) / (1.0 + counts[:, None])
    new_centers = (c64 - upd).astype(np.float32)
    loss = 0.5 * ((counts * (c64**2).sum(1)).sum() - 2.0 * (c64 * S).sum() + q)
    return np.float32(loss), new_centers


def kernel(preds, features, labels, centers):
    nc = _get_program()
    in_maps = []
    for k in range(NCORES):
        sl = slice(k * BS, (k + 1) * BS)
        in_maps.append(
            {
                "preds": np.ascontiguousarray(preds[sl], dtype=np.float32),
                "features": np.ascontiguousarray(features[sl], dtype=np.float32),
                "labels": np.ascontiguousarray(labels[sl], dtype=np.int32),
            }
        )
    res = run_bass_kernel_spmd(nc, in_maps, core_ids=list(range(NCORES)))
    parts = np.stack([res.results[k]["out"] for k in range(NCORES)])
    return combine_partials(parts, np.asarray(centers, dtype=np.float32))


# revision 16
# speedup vs baseline: 1.4224x; 1.0862x over previous
"""CenterCTCLoss Trainium2 kernel.

Strategy (data-parallel over batch, 8 cores, 512 rows each):
  The reference computation collapses to three segment statistics per core:
    counts[c] = sum of mask where labels==c
    S[c,d]    = sum of mask*features where labels==c
    SQ[c,d]   = sum of mask*features^2 where labels==c   (q = SQ.sum())
  where mask is the CTC char mask derived from argmax(preds).  Everything
  else (loss, centers update) is O(n_class*feat) math done on host from the
  8 per-core partial sums:
    upd[c]      = ALPHA*(counts[c]*centers[c] - S[c]) / (1+counts[c])
    new_centers = centers - upd
    loss        = 0.5*(sum_c counts[c]*|centers[c]|^2 - 2*<centers,S> + q)

  On device per chunk of 128 batch rows (batch on partitions):
    m[b,t]   = reduce_max over classes            (DVE)
    eq       = (preds == m)  exact one-hot        (DVE)
    sh       = eq[t]*eq[t+1] shifted product      (GPSIMD)
    rep[b,t] = sum_c sh                           (DVE)  == 1 iff argmax repeats
    mask     = (1-eq[...,BLANK])*(1-rep)          (tiny)
    moh      = one-hot(labels + 85*(1-mask)) bf16 (DVE)  masked one-hot
    psum[85,129] += moh_t.T @ [x_t | x_t^2 | 1]   (PE, bf16 ops, fp32 accum)
"""

import sys

sys.path.insert(0, "/opt/trn_rl_repo")

from contextlib import ExitStack

import numpy as np

import concourse.bacc as bacc
import concourse.bass as bass
import concourse.mybir as mybir
import concourse.tile as tile
from concourse.bass_utils import run_bass_kernel_spmd

N_CLASS = 85
BLANK = 84
ALPHA = 0.05
B, T, D = 4096, 96, 64
NCORES = 8
BS = B // NCORES  # 512 batch rows per core
TH = T // 2  # half-chunk along t for DMA tiles

f32 = mybir.dt.float32
i32 = mybir.dt.int32
i16 = mybir.dt.int16
bf16 = mybir.dt.bfloat16

FDP = T * N_CLASS  # 8160 preds elems per row
NOUT = 2 * D + 1  # 129 output cols: S | SQ | counts


def build_program(bs: int = BS):
    nchunk = bs // 128
    nc = bacc.Bacc(
        "TRN2", target_bir_lowering=False, debug=False, num_devices=NCORES
    )
    preds = nc.dram_tensor("preds", [bs, T, N_CLASS], f32, kind="ExternalInput").ap()
    feats = nc.dram_tensor("features", [bs, T, D], f32, kind="ExternalInput").ap()
    labels = nc.dram_tensor("labels", [bs, T], i32, kind="ExternalInput").ap()
    out = nc.dram_tensor("out", [N_CLASS, NOUT], f32, kind="ExternalOutput").ap()

    CP = N_CLASS + 1  # 86: class dim padded so bf16 shifted views stay 4B-aligned

    with tile.TileContext(nc, pool_alloc_mode="queue") as tc, ExitStack() as ctx:
        consts = ctx.enter_context(tc.tile_pool(name="consts", bufs=1))
        ppool = ctx.enter_context(tc.tile_pool(name="ppool", bufs=4))
        xpool = ctx.enter_context(tc.tile_pool(name="xpool", bufs=2))
        work = ctx.enter_context(tc.tile_pool(name="work", bufs=1))
        work2 = ctx.enter_context(tc.tile_pool(name="work2", bufs=2))
        mmin = ctx.enter_context(tc.tile_pool(name="mmin", bufs=2))
        small = ctx.enter_context(tc.tile_pool(name="small", bufs=4))
        psum_pool = ctx.enter_context(
            tc.tile_pool(name="psum", bufs=1, space="PSUM")
        )

        # per-t scatter constants: tb85[p,t] = (t%16)*85, jp1[p,t] = (t%16)+1
        TB = 16  # t-block size for local_scatter (16*85=1360 elems < 2047)
        NB = T // TB
        tb85i = consts.tile([128, T], i16)
        nc.gpsimd.iota(
            tb85i[:], pattern=[[0, NB], [N_CLASS, TB]], base=0, channel_multiplier=0
        )
        jp1i = consts.tile([128, T], i16)
        nc.gpsimd.iota(
            jp1i[:], pattern=[[0, NB], [1, TB]], base=1, channel_multiplier=0
        )
        tb85f = consts.tile([128, T], f32)
        nc.scalar.copy(tb85f[:], tb85i[:])
        jp1f = consts.tile([128, T], f32)
        nc.scalar.copy(jp1f[:], jp1i[:])
        ones16 = consts.tile([128, TB], bf16)
        nc.vector.memset(ones16[:], 1.0)

        psumA = psum_pool.tile([N_CLASS, D], f32)  # S
        psumB = psum_pool.tile([N_CLASS, D + 1], f32)  # SQ | counts

        def stage1(c):
            brange = slice(c * 128, (c + 1) * 128)
            # load; dd[p,t,c] = preds - max <= 0, exactly 0 at argmax
            dd = work2.tile([128, T, CP], bf16, tag="dd")
            m = small.tile([128, T], f32, tag="m")
            xb = mmin.tile([128, T, D], bf16, tag="xb")
            xsq = mmin.tile([128, T, D + 1], bf16, tag="xsq")
            labI = small.tile([128, T], i32, tag="labI")
            nc.sync.dma_start(labI[:], labels[brange, :])
            nc.gpsimd.memset(dd[:, :, N_CLASS], -1e30)
            TQ = T // 4
            for h in range(4):
                ts_ = slice(h * TQ, (h + 1) * TQ)
                ph = ppool.tile([128, TQ, N_CLASS], f32, tag="ph")
                nc.sync.dma_start(ph[:], preds[brange, ts_, :])

                nc.vector.tensor_reduce(
                    m[:, ts_], ph[:], axis=mybir.AxisListType.X, op=mybir.AluOpType.max
                )
                mb = m[:, ts_].unsqueeze(2).broadcast_to([128, TQ, N_CLASS])
                nc.gpsimd.tensor_tensor(
                    dd[:, ts_, 0:N_CLASS], ph[:], mb, op=mybir.AluOpType.subtract
                )
            for h in range(2):
                ts_ = slice(h * TH, (h + 1) * TH)
                xh = xpool.tile([128, TH, D], f32, tag="xh")
                nc.scalar.dma_start(xh[:], feats[brange, ts_, :])
                # features: bf16 copy and squares (ScalarE)
                nc.scalar.copy(xb[:, ts_, :], xh[:])
                nc.scalar.square(xsq[:, ts_, 0:D], xh[:])
            nc.vector.memset(xsq[:, :, D], 1.0)
            # blank-argmax indicator and scatter indices only need dd/labels
            inv84 = small.tile([128, T], f32, tag="inv84")
            nc.vector.tensor_single_scalar(
                inv84[:], dd[:, :, BLANK], 0.0, op=mybir.AluOpType.not_equal
            )
            labf = small.tile([128, T], f32, tag="labf")
            nc.scalar.copy(labf[:], labI[:])
            idx16 = small.tile([128, T], i16, tag="idx16")
            nc.vector.tensor_add(idx16[:], tb85f[:], labf[:])
            return dict(dd=dd, xb=xb, xsq=xsq, inv84=inv84, idx16=idx16)

        def stage2(c, st):
            dd, xb, xsq = st["dd"], st["xb"], st["xsq"]
            inv84, idx16 = st["inv84"], st["idx16"]
            ddf = dd[:].rearrange("p t c -> p (t c)")

            # repeated argmax: max_c(dd_t + dd_{t+1}) == 0 (exact)
            dsum = work.tile([128, (T - 1) * CP], bf16, tag="dsum")
            zmax = small.tile([128, T], f32, tag="zmax")
            nhalf = (T - 1) // 2  # 47; halves are 47 and 48 wide
            for w0, w1 in ((0, nhalf), (nhalf, T - 1)):
                nc.vector.tensor_add(
                    dsum[:, w0 * CP : w1 * CP],
                    ddf[:, w0 * CP : w1 * CP],
                    ddf[:, (w0 + 1) * CP : (w1 + 1) * CP],
                )
                nc.vector.tensor_reduce(
                    zmax[:, w0:w1],
                    dsum[:, w0 * CP : w1 * CP].rearrange("p (t c) -> p t c", c=CP),
                    axis=mybir.AxisListType.X,
                    op=mybir.AluOpType.max,
                )
            nc.vector.memset(zmax[:, T - 1 : T], -1.0)

            # mask = (dd[..,BLANK] != 0) * (zmax != 0), scattered as DATA:
            # moh[p, (t%16)*85+lab] = mask[p,t] (indices never masked, always
            # distinct within a 16-t block)
            invrep = small.tile([128, T], f32, tag="invrep")
            nc.vector.tensor_single_scalar(
                invrep[:], zmax[:], 0.0, op=mybir.AluOpType.not_equal
            )
            maskb = small.tile([128, T], bf16, tag="maskb")
            nc.vector.tensor_mul(maskb[:], invrep[:], inv84[:])

            # masked one-hot (bf16) via gpsimd local scatter
            moh = work.tile([128, FDP], bf16, tag="moh")
            moh3 = moh[:].rearrange("p (t c) -> p t c", c=N_CLASS)
            for blk in range(NB):
                nc.gpsimd.local_scatter(
                    moh[:, blk * TB * N_CLASS : (blk + 1) * TB * N_CLASS],
                    maskb[:, blk * TB : (blk + 1) * TB],
                    idx16[:, blk * TB : (blk + 1) * TB],
                    channels=128,
                    num_elems=TB * N_CLASS,
                    num_idxs=TB,
                )

            # segment sums via PE
            for t in range(T):
                first = c == 0 and t == 0
                last = c == nchunk - 1 and t == T - 1
                nc.tensor.matmul(
                    psumA[:], moh3[:, t, :], xb[:, t, :], start=first, stop=last
                )
                nc.tensor.matmul(
                    psumB[:], moh3[:, t, :], xsq[:, t, :], start=first, stop=last
                )

        # two-stage software pipeline: stage1(c+1) is emitted before
        # stage2(c) so GPSIMD/DMA of the next chunk overlap the mask/
        # scatter/matmul tail of the current one.
        pending = None
        for c in range(nchunk + 1):
            if c < nchunk:
                st = stage1(c)
            if pending is not None:
                stage2(c - 1, pending)
            pending = st if c < nchunk else None

        outT = consts.tile([N_CLASS, NOUT], f32)
        nc.scalar.copy(outT[:, 0:D], psumA[:])
        nc.scalar.copy(outT[:, D : 2 * D + 1], psumB[:])
        nc.sync.dma_start(out[:], outT[:])

    nc.compile()
    return nc


_prog_cache: dict[int, object] = {}


def _get_program(bs: int = BS):
    if bs not in _prog_cache:
        _prog_cache[bs] = build_program(bs)
    return _prog_cache[bs]


def combine_partials(parts: np.ndarray, centers: np.ndarray):
    """parts: [ncores, N_CLASS, NOUT] fp32 device partials -> (loss, new_centers)."""
    tot = parts.astype(np.float64).sum(axis=0)
    S = tot[:, 0:D]
    SQ = tot[:, D : 2 * D]
    counts = tot[:, 2 * D]
    q = SQ.sum()
    c64 = centers.astype(np.float64)
    upd = ALPHA * (counts[:, None] * c64 - S) / (1.0 + counts[:, None])
    new_centers = (c64 - upd).astype(np.float32)
    loss = 0.5 * ((counts * (c64**2).sum(1)).sum() - 2.0 * (c64 * S).sum() + q)
    return np.float32(loss), new_centers


def kernel(preds, features, labels, centers):
    nc = _get_program()
    in_maps = []
    for k in range(NCORES):
        sl = slice(k * BS, (k + 1) * BS)
        in_maps.append(
            {
                "preds": np.ascontiguousarray(preds[sl], dtype=np.float32),
                "features": np.ascontiguousarray(features[sl], dtype=np.float32),
                "labels": np.ascontiguousarray(labels[sl], dtype=np.int32),
            }
        )
    res = run_bass_kernel_spmd(nc, in_maps, core_ids=list(range(NCORES)))
    parts = np.stack([res.results[k]["out"] for k in range(NCORES)])
    return combine_partials(parts, np.asarray(centers, dtype=np.float32))
